# revision 21
# baseline (speedup 1.0000x reference)
"""Trainium2 Bass kernel for the ConvolutionalCapsule module (v5).

Sharding: data-parallel over (batch, H-half): core k handles b = k//2,
output rows h in [6*(k%2), 6*(k%2)+6), i.e. 72 spatial positions per core.
Weights are replicated. All FLOPs run on-device.

Design:
  - stage B: out0 = squash(mean_c preds) via K=(c,i)-chunked matmuls.
  - V[f,c,i] = sum_o W.out0 via octet-dense stationary weights w8o
    (K = (l8,o16) = 128 fully used), streaming a block-diagonal out0.
  - c-blocks 0/1 (full 128 c's): V evacuated PSUM->SBUF by the scalar
    engine into an i-major fp16 slab (contiguous writes); P-multiply is
    one 2x-mode DVE tensor_tensor per (octet, c-block) slab; i-sum is a
    DVE pair-tree (3 adds).
  - c-block 2 (32 c's): the 8 i-planes are packed 4-to-a-partition-block
    ((im, c) rows), so only 2 chunks per octet; the i-sum runs as tiny
    selection-matmuls on the PE with PSUM accumulation, exp reads PSUM
    directly. Kills 1/3 of the evac/VP/tree volume.
  - centroids via octet block-diagonal matmuls: lhsT = 128-col w_r
    slice, rhs = stacked S_f slabs; the (l,l') diagonal is extracted
    during PSUM evacuation via SBUF->SBUF DMAs (off-engine).
  - No GpSimd: it shares its SBUF port with the vector engine and
    measurably slows co-running DVE tensor_tensor ~4x.
"""
import numpy as np

KH = KW = 3
B, H, WD, FIN, DIN = 4, 14, 14, 32, 8
F, C, DO, DI = 32, 288, 16, 8
NPOS = 72
CBLK = 3
NCHUNK = DI * CBLK  # 24
NOCT = 4            # f-octets of 8
SLAB = 8 * NPOS     # 576 = (l, pos) extent per (octet, cblk)
SF = NCHUNK * NPOS  # 1728 = per-f S extent (i, cb, pos)
EPS = 1e-7

_CACHE: dict = {}


def _chunk_rows(t):
    i, cb = divmod(t, CBLK)
    c0 = cb * 128
    return i, c0, min(128, C - c0)


def _host_weights(Wm):
    """Wm: [F, C, DO, DI] float32 -> device weight layouts (fp16)."""
    w_r = np.zeros((NCHUNK, 128, F * DO), np.float16)
    for t in range(NCHUNK):
        i, c0, n = _chunk_rows(t)
        w_r[t, :n, :] = (
            Wm[:, c0:c0 + n, :, i].transpose(1, 0, 2).reshape(n, F * DO)
        )
    w_r = w_r.transpose(1, 0, 2).reshape(128, NCHUNK * F * DO).copy()
    # w8o[(l,o), (O,t2,c)]: octet-dense stationary V weights, c-blocks 0/1
    w8o = np.zeros((NOCT, 2 * DI, 128, 128), np.float16)
    for O in range(NOCT):
        for cb in range(2):
            for i in range(DI):
                c0 = cb * 128
                for l in range(8):
                    f = 8 * O + l
                    w8o[O, cb * DI + i, 16 * l:16 * l + DO, :] = \
                        Wm[f, c0:c0 + 128, :, i].T
    w8o = w8o.transpose(2, 0, 1, 3).reshape(128, NOCT * 2 * DI * 128).copy()
    # w8p[(l,o), (O,iq,(im,cc))]: cb=2 packed (4 i-planes per partition blk)
    w8p = np.zeros((NOCT, 2, 128, 128), np.float16)
    for O in range(NOCT):
        for iq in range(2):
            for im in range(4):
                i = 4 * iq + im
                for l in range(8):
                    f = 8 * O + l
                    w8p[O, iq, 16 * l:16 * l + DO, 32 * im:32 * im + 32] = \
                        Wm[f, 256:288, :, i].T
    w8p = w8p.transpose(2, 0, 1, 3).reshape(128, NOCT * 2 * 128).copy()
    # sel4[(im,cc), cc']: partition-sum selection matrix
    sel4 = np.zeros((128, 32), np.float16)
    for im in range(4):
        for cc in range(32):
            sel4[32 * im + cc, cc] = 1.0
    return w_r, w8o, w8p, sel4


def _host_patches(x, k):
    """Patch tensors for core k: p_ct [c,(i,cb,pos)], p_pk [(im,cc),(iq,pos)]."""
    b, hh = divmod(k, 2)
    h0 = 6 * hh
    P = np.empty((6, 12, KH, KW, FIN, DIN), np.float32)
    for kh in range(KH):
        for kw in range(KW):
            for h in range(6):
                P[h, :, kh, kw] = x[b, h0 + h + kh, kw:kw + 12]
    P = P.reshape(NPOS, C, DIN)
    p_ct = np.zeros((NCHUNK, 128, NPOS), np.float16)
    for t in range(NCHUNK):
        i, c0, n = _chunk_rows(t)
        p_ct[t, :n, :] = P[:, c0:c0 + n, i].T
    p_ct = p_ct.transpose(1, 0, 2).reshape(128, NCHUNK * NPOS).copy()
    p_pk = np.zeros((2, 128, NPOS), np.float16)
    for iq in range(2):
        for im in range(4):
            i = 4 * iq + im
            p_pk[iq, 32 * im:32 * im + 32, :] = P[:, 256:288, i].T
    p_pk = p_pk.transpose(1, 0, 2).reshape(128, 2 * NPOS).copy()
    return p_ct, p_pk


def _build():
    import concourse.bass as bass
    import concourse.bacc as bacc
    import concourse.mybir as mybir
    import concourse.tile as tile

    F16, F32 = mybir.dt.float16, mybir.dt.float32
    AX = mybir.AxisListType
    AF = mybir.ActivationFunctionType

    nc = bacc.Bacc(None, target_bir_lowering=False, debug=False)

    p_ct_d = nc.dram_tensor("p_ct", [128, NCHUNK * NPOS], F16, kind="ExternalInput")
    p_pk_d = nc.dram_tensor("p_pk", [128, 2 * NPOS], F16, kind="ExternalInput")
    w_r_d = nc.dram_tensor("w_r", [128, NCHUNK * F * DO], F16, kind="ExternalInput")
    w8o_d = nc.dram_tensor("w8o", [128, NOCT * 2 * DI * 128], F16, kind="ExternalInput")
    w8p_d = nc.dram_tensor("w8p", [128, NOCT * 2 * 128], F16, kind="ExternalInput")
    sel4_d = nc.dram_tensor("sel4", [128, 32], F16, kind="ExternalInput")
    eye72_d = nc.dram_tensor("eye72", [NPOS, NPOS], F32, kind="ExternalInput")
    eye128f_d = nc.dram_tensor("eye128f", [128, 128], F32, kind="ExternalInput")
    y_d = nc.dram_tensor("y", [NPOS, F * DO], F32, kind="ExternalOutput")

    with tile.TileContext(nc) as tc:
        with (
            tc.tile_pool(name="const", bufs=1) as const,
            tc.tile_pool(name="work", bufs=1) as work,
            tc.tile_pool(name="v16r", bufs=2) as v16r,
            tc.tile_pool(name="vpr", bufs=2) as vpr,
            tc.tile_pool(name="soct", bufs=2) as soct,
            tc.tile_pool(name="vps", bufs=2, space=bass.MemorySpace.PSUM) as vps,
            tc.tile_pool(name="agp", bufs=1, space=bass.MemorySpace.PSUM) as agp,
            tc.tile_pool(name="tps", bufs=1, space=bass.MemorySpace.PSUM) as tps,
            tc.tile_pool(name="bps", bufs=1, space=bass.MemorySpace.PSUM) as bps,
        ):
            # ---------------- loads (ordered for early stage-B start) -------
            p_ct = const.tile([128, NCHUNK * NPOS], F16, tag="p_ct")
            w_r = const.tile([128, NCHUNK * F * DO], F16, tag="w_r")
            # interleave patch/weight pieces so stage-B matmuls start early
            NG = 6
            PCG = NCHUNK * NPOS // NG
            WRG = NCHUNK * F * DO // NG
            for g in range(NG):
                nc.sync.dma_start(
                    p_ct[:, g * PCG:(g + 1) * PCG], p_ct_d[:, g * PCG:(g + 1) * PCG]
                )
                nc.sync.dma_start(
                    w_r[:, g * WRG:(g + 1) * WRG], w_r_d[:, g * WRG:(g + 1) * WRG]
                )
            eye72 = const.tile([NPOS, NPOS], F32, tag="eye72")
            nc.sync.dma_start(eye72[:], eye72_d[:])
            w8o = const.tile([128, NOCT * 2 * DI * 128], F16, tag="w8o")
            W8S = 2 * DI * 128
            for s in range(NOCT):
                nc.sync.dma_start(
                    w8o[:, s * W8S:(s + 1) * W8S], w8o_d[:, s * W8S:(s + 1) * W8S]
                )
            w8p = const.tile([128, NOCT * 2 * 128], F16, tag="w8p")
            nc.sync.dma_start(w8p[:], w8p_d[:])
            sel4 = const.tile([128, 32], F16, tag="sel4")
            nc.sync.dma_start(sel4[:], sel4_d[:])
            p_pk = const.tile([128, 2 * NPOS], F16, tag="p_pk")
            nc.sync.dma_start(p_pk[:], p_pk_d[:])
            eye128f = const.tile([128, 128], F32, tag="eye128f")
            nc.sync.dma_start(eye128f[:], eye128f_d[:])

            def squash(src_ap, dst_ap, pre_scale, tag, nf=F):
                """dst = squash(src * pre_scale) ; src free = (nf, DO)."""
                s = work.tile([NPOS, nf * DO], F32, tag=f"{tag}_s")
                sv = s[:].rearrange("p (f o) -> p f o", o=DO)
                nc.scalar.activation(s[:], src_ap, AF.Copy, scale=pre_scale)
                sq = work.tile([NPOS, nf * DO], F32, tag=f"{tag}_sq")
                nc.scalar.activation(sq[:], s[:], AF.Square)
                sn = work.tile([NPOS, nf], F32, tag=f"{tag}_sn")
                nc.vector.reduce_sum(
                    sn[:], sq[:].rearrange("p (f o) -> p f o", o=DO), axis=AX.X
                )
                t1 = work.tile([NPOS, nf], F32, tag=f"{tag}_t1")
                nc.vector.tensor_scalar_add(t1[:], sn[:], 1.0)
                r1 = work.tile([NPOS, nf], F32, tag=f"{tag}_r1")
                nc.vector.reciprocal(r1[:], t1[:])
                se = work.tile([NPOS, nf], F32, tag=f"{tag}_se")
                nc.vector.tensor_scalar_add(se[:], sn[:], EPS)
                # (sn+eps)^(-1/2) = exp(-0.5*ln(sn+eps)): keeps every scalar
                # activation in the one natural_log_exp_and_others table set
                # (a single ACT_TABLE_LOAD instead of four + mid-kernel swaps)
                r2 = work.tile([NPOS, nf], F32, tag=f"{tag}_r2")
                nc.scalar.activation(r2[:], se[:], AF.Ln)
                r3 = work.tile([NPOS, nf], F32, tag=f"{tag}_r3")
                nc.scalar.activation(r3[:], r2[:], AF.Exp, scale=-0.5)
                sc = work.tile([NPOS, nf], F32, tag=f"{tag}_sc")
                nc.vector.tensor_mul(sc[:], sn[:], r1[:])
                sc2 = work.tile([NPOS, nf], F32, tag=f"{tag}_sc2")
                nc.vector.tensor_mul(sc2[:], sc[:], r3[:])
                bc = sc2[:].unsqueeze(2).broadcast_to((NPOS, nf, DO))
                nc.vector.tensor_mul(dst_ap, sv, bc)

            # ---------------- stage B: out0 ----------------
            o0p = bps.tile([NPOS, F * DO], F32, tag="mm0")
            for t in range(NCHUNK):
                nc.tensor.matmul(
                    o0p[:],
                    p_ct[:, t * NPOS:(t + 1) * NPOS],
                    w_r[:, t * F * DO:(t + 1) * F * DO],
                    start=(t == 0),
                    stop=(t == NCHUNK - 1),
                )
            out0 = work.tile([NPOS, F * DO], F32, tag="out0")
            squash(
                o0p[:],
                out0[:].rearrange("p (f o) -> p f o", o=DO),
                1.0 / F,
                "sq1",
            )

            # transposes -> bd[(l,o), (O; l,pos)] block-diagonal (fp16)
            bd = work.tile([128, NOCT * SLAB], F16, tag="bd")
            nc.vector.memset(bd[:], 0.0)
            # e16[c; (O, cb, l, pos)]; memset to 1.0 so cb=2 dead rows give
            # finite Z (pp = 0 * (1/Z) stays 0, no NaN)
            e16 = work.tile([128, NOCT * CBLK * SLAB], F16, tag="e16")
            nc.vector.memset(e16[:], 1.0)
            for O in range(NOCT):
                tp = tps.tile([128, 128], F32, tag="tp")
                nc.tensor.transpose(
                    tp[:, 0:NPOS], out0[:, O * 128:(O + 1) * 128], eye72[:]
                )
                tpq = work.tile([128, NPOS], F16, tag=f"tpq{O}")
                nc.scalar.copy(tpq[:], tp[:, 0:NPOS])
                for l in range(8):
                    # strips sit at 16-mod-32 partition bases, which compute
                    # engines cannot address; DMA keeps them off the DVE too
                    nc.sync.dma_start(
                        bd[16 * l:16 * l + DO,
                           O * SLAB + l * NPOS:O * SLAB + (l + 1) * NPOS],
                        tpq[16 * l:16 * l + DO, :],
                    )

            # ---------------- V + VP + agr + exp ----------------
            zA = work.tile([128, CBLK * SLAB], F16, tag="zA")
            zB = work.tile([128, CBLK * SLAB], F16, tag="zB")
            pcv = p_ct[:].rearrange("p (i cb n) -> p i cb n", i=DI, cb=CBLK)

            for O in range(NOCT):
                # --- c-blocks 0/1: full slabs, scalar evac + DVE VP/tree ---
                for cb in range(2):
                    s = O * CBLK + cb
                    vp = vpr.tile([128, DI * SLAB], F16, tag="vp")
                    v16 = v16r.tile([128, DI * SLAB], F16, tag="v16")
                    for i in range(DI):
                        vh = vps.tile([128, 1024], F32, tag="vh")
                        for h in range(2):
                            nc.tensor.matmul(
                                vh[:, 512 * h:512 * h + 4 * NPOS],
                                w8o[:, (O * 2 * DI + cb * DI + i) * 128:
                                    (O * 2 * DI + cb * DI + i + 1) * 128],
                                bd[:, O * SLAB + h * 4 * NPOS:
                                   O * SLAB + (h + 1) * 4 * NPOS],
                                start=True,
                                stop=True,
                            )
                        nc.scalar.copy(
                            v16[:, i * SLAB:(i + 1) * SLAB]
                            .rearrange("p (h l n) -> p h l n", h=2, l=4),
                            vh[:].rearrange("p (h x) -> p h x", h=2)
                            [:, :, 0:4 * NPOS]
                            .rearrange("p h (l n) -> p h l n", l=4),
                        )
                    nc.vector.tensor_mul(
                        vp[:].rearrange("p (i l n) -> p i l n", i=DI, l=8),
                        v16[:].rearrange("p (i l n) -> p i l n", i=DI, l=8),
                        pcv[:, :, cb, :].unsqueeze(2)
                        .broadcast_to((128, DI, 8, NPOS)),
                    )
                    tr1 = work.tile([128, 4 * SLAB], F16, tag="tr1")
                    nc.vector.tensor_add(
                        tr1[:], vp[:, 0:4 * SLAB], vp[:, 4 * SLAB:8 * SLAB]
                    )
                    tr2 = work.tile([128, 2 * SLAB], F16, tag="tr2")
                    nc.vector.tensor_add(
                        tr2[:], tr1[:, 0:2 * SLAB], tr1[:, 2 * SLAB:4 * SLAB]
                    )
                    agr = work.tile([128, SLAB], F16, tag="agr")
                    nc.vector.tensor_add(
                        agr[:], tr2[:, 0:SLAB], tr2[:, SLAB:2 * SLAB]
                    )
                    nc.scalar.activation(
                        e16[:, s * SLAB:(s + 1) * SLAB], agr[:], AF.Exp
                    )
                # --- c-block 2: packed (im, cc) rows, PE sel-reduce ---
                vpk = vpr.tile([128, 2 * SLAB], F16, tag="vpk")
                for iq in range(2):
                    vh = vps.tile([128, 1024], F32, tag="vh")
                    for h in range(2):
                        nc.tensor.matmul(
                            vh[:, 512 * h:512 * h + 4 * NPOS],
                            w8p[:, (O * 2 + iq) * 128:(O * 2 + iq + 1) * 128],
                            bd[:, O * SLAB + h * 4 * NPOS:
                               O * SLAB + (h + 1) * 4 * NPOS],
                            start=True,
                            stop=True,
                        )
                    # fused evac * P on DVE (PSUM 1x path, small)
                    pb = (
                        p_pk[:].rearrange("p (iq n) -> p iq n", iq=2)[:, iq, :]
                        .unsqueeze(1).unsqueeze(1)
                        .broadcast_to((128, 2, 4, NPOS))
                    )
                    nc.vector.tensor_mul(
                        vpk[:, iq * SLAB:(iq + 1) * SLAB]
                        .rearrange("p (h l n) -> p h l n", h=2, l=4),
                        vh[:].rearrange("p (h x) -> p h x", h=2)
                        [:, :, 0:4 * NPOS]
                        .rearrange("p h (l n) -> p h l n", l=4),
                        pb,
                    )
                # agr_cb2[cc, (h,l,n)] = sum_im sum_iq vpk  (PE sel-matmul)
                ag2 = agp.tile([32, 1024], F32, tag="ag2")
                for h in range(2):
                    for iq in range(2):
                        nc.tensor.matmul(
                            ag2[:, 512 * h:512 * h + 4 * NPOS],
                            sel4[:],
                            vpk[:, iq * SLAB + h * 4 * NPOS:
                                iq * SLAB + (h + 1) * 4 * NPOS],
                            start=(iq == 0),
                            stop=(iq == 1),
                        )
                nc.scalar.activation(
                    e16[0:32, (O * CBLK + 2) * SLAB:(O * CBLK + 3) * SLAB]
                    .rearrange("p (h x) -> p h x", h=2),
                    ag2[:].rearrange("p (h x) -> p h x", h=2)[:, :, 0:4 * NPOS],
                    AF.Exp,
                )
                # incremental softmax-normalizer partials (over octet pairs)
                if O == 1:
                    nc.vector.tensor_add(
                        zA[:], e16[:, 0:CBLK * SLAB],
                        e16[:, CBLK * SLAB:2 * CBLK * SLAB],
                    )
                if O == 3:
                    nc.vector.tensor_add(
                        zB[:], e16[:, 2 * CBLK * SLAB:3 * CBLK * SLAB],
                        e16[:, 3 * CBLK * SLAB:4 * CBLK * SLAB],
                    )

            # ---------------- Z l-tree + pp ----------------
            zAB = work.tile([128, CBLK * SLAB], F16, tag="zAB")
            nc.vector.tensor_add(zAB[:], zA[:], zB[:])
            zv = zAB[:].rearrange("p (cb l n) -> p cb l n", cb=CBLK, l=8)
            zt1 = work.tile([128, CBLK * 4 * NPOS], F16, tag="zt1")
            nc.vector.tensor_add(
                zt1[:].rearrange("p (cb l n) -> p cb l n", cb=CBLK, l=4),
                zv[:, :, 0:4], zv[:, :, 4:8],
            )
            z1 = zt1[:].rearrange("p (cb l n) -> p cb l n", cb=CBLK, l=4)
            zt2 = work.tile([128, CBLK * 2 * NPOS], F16, tag="zt2")
            nc.vector.tensor_add(
                zt2[:].rearrange("p (cb l n) -> p cb l n", cb=CBLK, l=2),
                z1[:, :, 0:2], z1[:, :, 2:4],
            )
            z2 = zt2[:].rearrange("p (cb l n) -> p cb l n", cb=CBLK, l=2)
            Zf = work.tile([128, CBLK * NPOS], F32, tag="Zf")
            nc.vector.tensor_add(
                Zf[:].rearrange("p (cb n) -> p cb n", cb=CBLK),
                z2[:, :, 0], z2[:, :, 1],
            )
            Zr = work.tile([128, CBLK * NPOS], F16, tag="Zr")
            with nc.allow_low_precision(reason="Z in [F/e, F*e]; fp16 1/Z fine"):
                nc.vector.reciprocal(Zr[:], Zf[:])
            pp = work.tile([128, NCHUNK * NPOS], F16, tag="pp")
            nc.vector.tensor_mul(
                pp[:].rearrange("p (i cb n) -> p i cb n", i=DI, cb=CBLK),
                pcv,
                Zr[:].rearrange("p (cb n) -> p cb n", cb=CBLK)
                .unsqueeze(1)
                .broadcast_to((128, DI, CBLK, NPOS)),
            )

            # ---------------- S + centroids (octet block-diag) -------------
            opre = work.tile([NPOS, F * DO], F32, tag="opre")
            y_sb = work.tile([NPOS, F * DO], F32, tag="y_sb")
            ppv = pp[:].rearrange("p (i cb n) -> p i cb n", i=DI, cb=CBLK)
            for O in range(NOCT):
                So = soct.tile([128, 8 * SF], F16, tag="So")
                for l in range(8):
                    eb = (
                        e16[:, O * CBLK * SLAB:(O + 1) * CBLK * SLAB]
                        .rearrange("p (cb l n) -> p cb l n", cb=CBLK, l=8)
                        [:, :, l, :]
                        .unsqueeze(1)
                        .broadcast_to((128, DI, CBLK, NPOS))
                    )
                    nc.vector.tensor_mul(
                        So[:, l * SF:(l + 1) * SF]
                        .rearrange("p (i cb n) -> p i cb n", i=DI, cb=CBLK),
                        ppv,
                        eb,
                    )
                cm = vps.tile([128, 1024], F32, tag="vh")
                Sv = So[:].rearrange("p (l x) -> p l x", l=8)
                for t in range(NCHUNK):
                    lhsT = w_r[:, t * F * DO + 128 * O:t * F * DO + 128 * (O + 1)]
                    for h in range(2):
                        nc.tensor.matmul(
                            cm[:, 512 * h:512 * h + 4 * NPOS]
                            .rearrange("p (l n) -> p l n", l=4),
                            lhsT,
                            Sv[:, 4 * h:4 * h + 4,
                               t * NPOS:(t + 1) * NPOS],
                            start=(t == 0),
                            stop=(t == NCHUNK - 1),
                        )
                # diagonal extraction: cen_oct[(l,o), pos] = cm[(l,o), (l,pos)]
                cmsb = work.tile([128, 8 * NPOS], F32, tag="cmsb")
                nc.scalar.copy(
                    cmsb[:].rearrange("p (h x) -> p h x", h=2),
                    cm[:].rearrange("p (h x) -> p h x", h=2)[:, :, 0:4 * NPOS],
                )
                cen_oct = work.tile([128, NPOS], F32, tag=f"cen{O}")
                for l in range(8):
                    src = cmsb[16 * l:16 * l + DO, l * NPOS:(l + 1) * NPOS]
                    dst = cen_oct[16 * l:16 * l + DO, :]
                    if l % 2 == 0:
                        nc.vector.tensor_copy(dst, src)
                    else:
                        nc.sync.dma_start(dst, src)
                tp2 = tps.tile([128, 128], F32, tag="tp")
                nc.tensor.transpose(tp2[0:NPOS, :], cen_oct[:], eye128f[:])
                nc.scalar.copy(opre[:, O * 128:(O + 1) * 128], tp2[0:NPOS, :])
                # per-octet squash2 (overlaps next octet's S/centroids)
                squash(
                    opre[:, O * 128:(O + 1) * 128]
                    .rearrange("p (f o) -> p f o", o=DO),
                    y_sb[:, O * 128:(O + 1) * 128]
                    .rearrange("p (f o) -> p f o", o=DO),
                    1.0,
                    f"sq2_{O}",
                    nf=8,
                )
                nc.sync.dma_start(
                    y_d[:, O * 128:(O + 1) * 128], y_sb[:, O * 128:(O + 1) * 128]
                )

    nc.compile()
    return nc


def _get_program():
    if "nc" not in _CACHE:
        _CACHE["nc"] = _build()
    return _CACHE["nc"]


def _in_maps(x, Wm):
    w_r, w8o, w8p, sel4 = _host_weights(Wm)
    eye72 = np.eye(NPOS, dtype=np.float32)
    eye128f = np.eye(128, dtype=np.float32)
    maps = []
    for k in range(8):
        p_ct, p_pk = _host_patches(x, k)
        maps.append({
            "p_ct": p_ct,
            "p_pk": p_pk,
            "w_r": w_r,
            "w8o": w8o,
            "w8p": w8p,
            "sel4": sel4,
            "eye72": eye72,
            "eye128f": eye128f,
        })
    return maps


def kernel(x, W):
    from concourse.bass_utils import run_bass_kernel_spmd

    x = np.asarray(x, np.float32)
    Wm = np.asarray(W, np.float32)[0, 0, 0]
    nc = _get_program()
    res = run_bass_kernel_spmd(nc, _in_maps(x, Wm), list(range(8)))
    Ho, Wo = H - KH + 1, WD - KW + 1
    y = np.empty((B, Ho, Wo, F, DO), np.float32)
    for k in range(8):
        b, hh = divmod(k, 2)
        y[b, 6 * hh:6 * hh + 6] = res.results[k]["y"].reshape(6, Wo, F, DO)
    return y


# revision 22
# speedup vs baseline: 1.0405x; 1.0405x over previous
"""Trainium2 Bass kernel for the ConvolutionalCapsule module (v5).

Sharding: data-parallel over (batch, H-half): core k handles b = k//2,
output rows h in [6*(k%2), 6*(k%2)+6), i.e. 72 spatial positions per core.
Weights are replicated. All FLOPs run on-device.

Design:
  - stage B: out0 = squash(mean_c preds) via K=(c,i)-chunked matmuls.
  - V[f,c,i] = sum_o W.out0 via octet-dense stationary weights w8o
    (K = (l8,o16) = 128 fully used), streaming a block-diagonal out0.
  - c-blocks 0/1 (full 128 c's): V evacuated PSUM->SBUF by the scalar
    engine into an i-major fp16 slab (contiguous writes); P-multiply is
    one 2x-mode DVE tensor_tensor per (octet, c-block) slab; i-sum is a
    DVE pair-tree (3 adds).
  - c-block 2 (32 c's): the 8 i-planes are packed 4-to-a-partition-block
    ((im, c) rows), so only 2 chunks per octet; the i-sum runs as tiny
    selection-matmuls on the PE with PSUM accumulation, exp reads PSUM
    directly. Kills 1/3 of the evac/VP/tree volume.
  - centroids via octet block-diagonal matmuls: lhsT = 128-col w_r
    slice, rhs = stacked S_f slabs; the (l,l') diagonal is extracted
    during PSUM evacuation via SBUF->SBUF DMAs (off-engine).
  - No GpSimd: it shares its SBUF port with the vector engine and
    measurably slows co-running DVE tensor_tensor ~4x.
"""
import numpy as np

KH = KW = 3
B, H, WD, FIN, DIN = 4, 14, 14, 32, 8
F, C, DO, DI = 32, 288, 16, 8
NPOS = 72
CBLK = 3
NCHUNK = DI * CBLK  # 24
NOCT = 4            # f-octets of 8
SLAB = 8 * NPOS     # 576 = (l, pos) extent per (octet, cblk)
SF = NCHUNK * NPOS  # 1728 = per-f S extent (i, cb, pos)
EPS = 1e-7

_CACHE: dict = {}


def _chunk_rows(t):
    i, cb = divmod(t, CBLK)
    c0 = cb * 128
    return i, c0, min(128, C - c0)


def _host_weights(Wm):
    """Wm: [F, C, DO, DI] float32 -> device weight layouts (fp16)."""
    w_r = np.zeros((NCHUNK, 128, F * DO), np.float16)
    for t in range(NCHUNK):
        i, c0, n = _chunk_rows(t)
        w_r[t, :n, :] = (
            Wm[:, c0:c0 + n, :, i].transpose(1, 0, 2).reshape(n, F * DO)
        )
    w_r = w_r.transpose(1, 0, 2).reshape(128, NCHUNK * F * DO).copy()
    # w8o[(l,o), (O,t2,c)]: octet-dense stationary V weights, c-blocks 0/1
    w8o = np.zeros((NOCT, 2 * DI, 128, 128), np.float16)
    for O in range(NOCT):
        for cb in range(2):
            for i in range(DI):
                c0 = cb * 128
                for l in range(8):
                    f = 8 * O + l
                    w8o[O, cb * DI + i, 16 * l:16 * l + DO, :] = \
                        Wm[f, c0:c0 + 128, :, i].T
    w8o = w8o.transpose(2, 0, 1, 3).reshape(128, NOCT * 2 * DI * 128).copy()
    # w8p[(l,o), (O,iq,(im,cc))]: cb=2 packed (4 i-planes per partition blk)
    w8p = np.zeros((NOCT, 2, 128, 128), np.float16)
    for O in range(NOCT):
        for iq in range(2):
            for im in range(4):
                i = 4 * iq + im
                for l in range(8):
                    f = 8 * O + l
                    w8p[O, iq, 16 * l:16 * l + DO, 32 * im:32 * im + 32] = \
                        Wm[f, 256:288, :, i].T
    w8p = w8p.transpose(2, 0, 1, 3).reshape(128, NOCT * 2 * 128).copy()
    # sel4[(im,cc), cc']: partition-sum selection matrix
    sel4 = np.zeros((128, 32), np.float16)
    for im in range(4):
        for cc in range(32):
            sel4[32 * im + cc, cc] = 1.0
    return w_r, w8o, w8p, sel4


def _host_patches(x, k):
    """Patch tensors for core k: p_ct [c,(i,cb,pos)], p_pk [(im,cc),(iq,pos)]."""
    b, hh = divmod(k, 2)
    h0 = 6 * hh
    P = np.empty((6, 12, KH, KW, FIN, DIN), np.float32)
    for kh in range(KH):
        for kw in range(KW):
            for h in range(6):
                P[h, :, kh, kw] = x[b, h0 + h + kh, kw:kw + 12]
    P = P.reshape(NPOS, C, DIN)
    p_ct = np.zeros((NCHUNK, 128, NPOS), np.float16)
    for t in range(NCHUNK):
        i, c0, n = _chunk_rows(t)
        p_ct[t, :n, :] = P[:, c0:c0 + n, i].T
    p_ct = p_ct.transpose(1, 0, 2).reshape(128, NCHUNK * NPOS).copy()
    p_pk = np.zeros((2, 128, NPOS), np.float16)
    for iq in range(2):
        for im in range(4):
            i = 4 * iq + im
            p_pk[iq, 32 * im:32 * im + 32, :] = P[:, 256:288, i].T
    p_pk = p_pk.transpose(1, 0, 2).reshape(128, 2 * NPOS).copy()
    return p_ct, p_pk


def _build():
    import concourse.bass as bass
    import concourse.bacc as bacc
    import concourse.mybir as mybir
    import concourse.tile as tile

    F16, F32 = mybir.dt.float16, mybir.dt.float32
    AX = mybir.AxisListType
    AF = mybir.ActivationFunctionType

    nc = bacc.Bacc(None, target_bir_lowering=False, debug=False)

    p_ct_d = nc.dram_tensor("p_ct", [128, NCHUNK * NPOS], F16, kind="ExternalInput")
    p_pk_d = nc.dram_tensor("p_pk", [128, 2 * NPOS], F16, kind="ExternalInput")
    w_r_d = nc.dram_tensor("w_r", [128, NCHUNK * F * DO], F16, kind="ExternalInput")
    w8o_d = nc.dram_tensor("w8o", [128, NOCT * 2 * DI * 128], F16, kind="ExternalInput")
    w8p_d = nc.dram_tensor("w8p", [128, NOCT * 2 * 128], F16, kind="ExternalInput")
    sel4_d = nc.dram_tensor("sel4", [128, 32], F16, kind="ExternalInput")
    eye72_d = nc.dram_tensor("eye72", [NPOS, NPOS], F32, kind="ExternalInput")
    eye128f_d = nc.dram_tensor("eye128f", [128, 128], F32, kind="ExternalInput")
    y_d = nc.dram_tensor("y", [NPOS, F * DO], F32, kind="ExternalOutput")

    with tile.TileContext(nc) as tc:
        with (
            tc.tile_pool(name="const", bufs=1) as const,
            tc.tile_pool(name="work", bufs=1) as work,
            tc.tile_pool(name="v16r", bufs=2) as v16r,
            tc.tile_pool(name="vpr", bufs=2) as vpr,
            tc.tile_pool(name="soct", bufs=2) as soct,
            tc.tile_pool(name="vps", bufs=2, space=bass.MemorySpace.PSUM) as vps,
            tc.tile_pool(name="agp", bufs=1, space=bass.MemorySpace.PSUM) as agp,
            tc.tile_pool(name="tps", bufs=1, space=bass.MemorySpace.PSUM) as tps,
            tc.tile_pool(name="bps", bufs=1, space=bass.MemorySpace.PSUM) as bps,
        ):
            # ---------------- loads (ordered for early stage-B start) -------
            p_ct = const.tile([128, NCHUNK * NPOS], F16, tag="p_ct")
            w_r = const.tile([128, NCHUNK * F * DO], F16, tag="w_r")
            # interleave patch/weight pieces so stage-B matmuls start early
            NG = 6
            PCG = NCHUNK * NPOS // NG
            WRG = NCHUNK * F * DO // NG
            for g in range(NG):
                nc.sync.dma_start(
                    p_ct[:, g * PCG:(g + 1) * PCG], p_ct_d[:, g * PCG:(g + 1) * PCG]
                )
                nc.sync.dma_start(
                    w_r[:, g * WRG:(g + 1) * WRG], w_r_d[:, g * WRG:(g + 1) * WRG]
                )
            eye72 = const.tile([NPOS, NPOS], F32, tag="eye72")
            nc.sync.dma_start(eye72[:], eye72_d[:])
            w8o = const.tile([128, NOCT * 2 * DI * 128], F16, tag="w8o")
            W8S = 2 * DI * 128
            for s in range(NOCT):
                nc.sync.dma_start(
                    w8o[:, s * W8S:(s + 1) * W8S], w8o_d[:, s * W8S:(s + 1) * W8S]
                )
            w8p = const.tile([128, NOCT * 2 * 128], F16, tag="w8p")
            nc.sync.dma_start(w8p[:], w8p_d[:])
            sel4 = const.tile([128, 32], F16, tag="sel4")
            nc.sync.dma_start(sel4[:], sel4_d[:])
            p_pk = const.tile([128, 2 * NPOS], F16, tag="p_pk")
            nc.sync.dma_start(p_pk[:], p_pk_d[:])
            eye128f = const.tile([128, 128], F32, tag="eye128f")
            nc.sync.dma_start(eye128f[:], eye128f_d[:])

            def squash(src_ap, dst_ap, pre_scale, tag, nf=F):
                """dst = squash(src * pre_scale) ; src free = (nf, DO)."""
                s = work.tile([NPOS, nf * DO], F32, tag=f"{tag}_s")
                sv = s[:].rearrange("p (f o) -> p f o", o=DO)
                nc.scalar.activation(s[:], src_ap, AF.Copy, scale=pre_scale)
                sq = work.tile([NPOS, nf * DO], F32, tag=f"{tag}_sq")
                nc.scalar.activation(sq[:], s[:], AF.Square)
                sn = work.tile([NPOS, nf], F32, tag=f"{tag}_sn")
                nc.vector.reduce_sum(
                    sn[:], sq[:].rearrange("p (f o) -> p f o", o=DO), axis=AX.X
                )
                t1 = work.tile([NPOS, nf], F32, tag=f"{tag}_t1")
                nc.vector.tensor_scalar_add(t1[:], sn[:], 1.0)
                r1 = work.tile([NPOS, nf], F32, tag=f"{tag}_r1")
                nc.vector.reciprocal(r1[:], t1[:])
                se = work.tile([NPOS, nf], F32, tag=f"{tag}_se")
                nc.vector.tensor_scalar_add(se[:], sn[:], EPS)
                r2 = work.tile([NPOS, nf], F32, tag=f"{tag}_r2")
                nc.scalar.activation(r2[:], se[:], AF.Sqrt)
                r3 = work.tile([NPOS, nf], F32, tag=f"{tag}_r3")
                nc.vector.reciprocal(r3[:], r2[:])
                sc = work.tile([NPOS, nf], F32, tag=f"{tag}_sc")
                nc.vector.tensor_mul(sc[:], sn[:], r1[:])
                sc2 = work.tile([NPOS, nf], F32, tag=f"{tag}_sc2")
                nc.vector.tensor_mul(sc2[:], sc[:], r3[:])
                bc = sc2[:].unsqueeze(2).broadcast_to((NPOS, nf, DO))
                nc.vector.tensor_mul(dst_ap, sv, bc)

            # ---------------- stage B: out0 ----------------
            o0p = bps.tile([NPOS, F * DO], F32, tag="mm0")
            for t in range(NCHUNK):
                nc.tensor.matmul(
                    o0p[:],
                    p_ct[:, t * NPOS:(t + 1) * NPOS],
                    w_r[:, t * F * DO:(t + 1) * F * DO],
                    start=(t == 0),
                    stop=(t == NCHUNK - 1),
                )
            out0 = work.tile([NPOS, F * DO], F32, tag="out0")
            squash(
                o0p[:],
                out0[:].rearrange("p (f o) -> p f o", o=DO),
                1.0 / F,
                "sq1",
            )

            # transposes -> bd[(l,o), (O; l,pos)] block-diagonal (fp16)
            bd = work.tile([128, NOCT * SLAB], F16, tag="bd")
            nc.vector.memset(bd[:], 0.0)
            # e16[c; (O, cb, l, pos)]; memset to 1.0 so cb=2 dead rows give
            # finite Z (pp = 0 * (1/Z) stays 0, no NaN)
            e16 = work.tile([128, NOCT * CBLK * SLAB], F16, tag="e16")
            nc.vector.memset(e16[:], 1.0)
            for O in range(NOCT):
                tp = tps.tile([128, 128], F32, tag="tp")
                nc.tensor.transpose(
                    tp[:, 0:NPOS], out0[:, O * 128:(O + 1) * 128], eye72[:]
                )
                tpq = work.tile([128, NPOS], F16, tag=f"tpq{O}")
                nc.scalar.copy(tpq[:], tp[:, 0:NPOS])
                for l in range(8):
                    # strips sit at 16-mod-32 partition bases, which compute
                    # engines cannot address; DMA keeps them off the DVE too
                    nc.sync.dma_start(
                        bd[16 * l:16 * l + DO,
                           O * SLAB + l * NPOS:O * SLAB + (l + 1) * NPOS],
                        tpq[16 * l:16 * l + DO, :],
                    )

            # ---------------- V + VP + agr + exp ----------------
            zA = work.tile([128, CBLK * SLAB], F16, tag="zA")
            zB = work.tile([128, CBLK * SLAB], F16, tag="zB")
            pcv = p_ct[:].rearrange("p (i cb n) -> p i cb n", i=DI, cb=CBLK)

            for O in range(NOCT):
                # --- c-blocks 0/1: full slabs, scalar evac + DVE VP/tree ---
                for cb in range(2):
                    s = O * CBLK + cb
                    vp = vpr.tile([128, DI * SLAB], F16, tag="vp")
                    v16 = v16r.tile([128, DI * SLAB], F16, tag="v16")
                    for i in range(DI):
                        vh = vps.tile([128, 1024], F32, tag="vh")
                        for h in range(2):
                            nc.tensor.matmul(
                                vh[:, 512 * h:512 * h + 4 * NPOS],
                                w8o[:, (O * 2 * DI + cb * DI + i) * 128:
                                    (O * 2 * DI + cb * DI + i + 1) * 128],
                                bd[:, O * SLAB + h * 4 * NPOS:
                                   O * SLAB + (h + 1) * 4 * NPOS],
                                start=True,
                                stop=True,
                            )
                        nc.scalar.copy(
                            v16[:, i * SLAB:(i + 1) * SLAB]
                            .rearrange("p (h l n) -> p h l n", h=2, l=4),
                            vh[:].rearrange("p (h x) -> p h x", h=2)
                            [:, :, 0:4 * NPOS]
                            .rearrange("p h (l n) -> p h l n", l=4),
                        )
                    nc.vector.tensor_mul(
                        vp[:].rearrange("p (i l n) -> p i l n", i=DI, l=8),
                        v16[:].rearrange("p (i l n) -> p i l n", i=DI, l=8),
                        pcv[:, :, cb, :].unsqueeze(2)
                        .broadcast_to((128, DI, 8, NPOS)),
                    )
                    tr1 = work.tile([128, 4 * SLAB], F16, tag="tr1")
                    nc.vector.tensor_add(
                        tr1[:], vp[:, 0:4 * SLAB], vp[:, 4 * SLAB:8 * SLAB]
                    )
                    tr2 = work.tile([128, 2 * SLAB], F16, tag="tr2")
                    nc.vector.tensor_add(
                        tr2[:], tr1[:, 0:2 * SLAB], tr1[:, 2 * SLAB:4 * SLAB]
                    )
                    agr = work.tile([128, SLAB], F16, tag="agr")
                    nc.vector.tensor_add(
                        agr[:], tr2[:, 0:SLAB], tr2[:, SLAB:2 * SLAB]
                    )
                    nc.scalar.activation(
                        e16[:, s * SLAB:(s + 1) * SLAB], agr[:], AF.Exp
                    )
                # --- c-block 2: packed (im, cc) rows, PE sel-reduce ---
                vpk = vpr.tile([128, 2 * SLAB], F16, tag="vpk")
                for iq in range(2):
                    vh = vps.tile([128, 1024], F32, tag="vh")
                    for h in range(2):
                        nc.tensor.matmul(
                            vh[:, 512 * h:512 * h + 4 * NPOS],
                            w8p[:, (O * 2 + iq) * 128:(O * 2 + iq + 1) * 128],
                            bd[:, O * SLAB + h * 4 * NPOS:
                               O * SLAB + (h + 1) * 4 * NPOS],
                            start=True,
                            stop=True,
                        )
                    # fused evac * P on DVE (PSUM 1x path, small)
                    pb = (
                        p_pk[:].rearrange("p (iq n) -> p iq n", iq=2)[:, iq, :]
                        .unsqueeze(1).unsqueeze(1)
                        .broadcast_to((128, 2, 4, NPOS))
                    )
                    nc.vector.tensor_mul(
                        vpk[:, iq * SLAB:(iq + 1) * SLAB]
                        .rearrange("p (h l n) -> p h l n", h=2, l=4),
                        vh[:].rearrange("p (h x) -> p h x", h=2)
                        [:, :, 0:4 * NPOS]
                        .rearrange("p h (l n) -> p h l n", l=4),
                        pb,
                    )
                # agr_cb2[cc, (h,l,n)] = sum_im sum_iq vpk  (PE sel-matmul)
                ag2 = agp.tile([32, 1024], F32, tag="ag2")
                for h in range(2):
                    for iq in range(2):
                        nc.tensor.matmul(
                            ag2[:, 512 * h:512 * h + 4 * NPOS],
                            sel4[:],
                            vpk[:, iq * SLAB + h * 4 * NPOS:
                                iq * SLAB + (h + 1) * 4 * NPOS],
                            start=(iq == 0),
                            stop=(iq == 1),
                        )
                nc.scalar.activation(
                    e16[0:32, (O * CBLK + 2) * SLAB:(O * CBLK + 3) * SLAB]
                    .rearrange("p (h x) -> p h x", h=2),
                    ag2[:].rearrange("p (h x) -> p h x", h=2)[:, :, 0:4 * NPOS],
                    AF.Exp,
                )
                # incremental softmax-normalizer partials (over octet pairs)
                if O == 1:
                    nc.vector.tensor_add(
                        zA[:], e16[:, 0:CBLK * SLAB],
                        e16[:, CBLK * SLAB:2 * CBLK * SLAB],
                    )
                if O == 3:
                    nc.vector.tensor_add(
                        zB[:], e16[:, 2 * CBLK * SLAB:3 * CBLK * SLAB],
                        e16[:, 3 * CBLK * SLAB:4 * CBLK * SLAB],
                    )

            # ---------------- Z l-tree + pp ----------------
            zAB = work.tile([128, CBLK * SLAB], F16, tag="zAB")
            nc.vector.tensor_add(zAB[:], zA[:], zB[:])
            zv = zAB[:].rearrange("p (cb l n) -> p cb l n", cb=CBLK, l=8)
            zt1 = work.tile([128, CBLK * 4 * NPOS], F16, tag="zt1")
            nc.vector.tensor_add(
                zt1[:].rearrange("p (cb l n) -> p cb l n", cb=CBLK, l=4),
                zv[:, :, 0:4], zv[:, :, 4:8],
            )
            z1 = zt1[:].rearrange("p (cb l n) -> p cb l n", cb=CBLK, l=4)
            zt2 = work.tile([128, CBLK * 2 * NPOS], F16, tag="zt2")
            nc.vector.tensor_add(
                zt2[:].rearrange("p (cb l n) -> p cb l n", cb=CBLK, l=2),
                z1[:, :, 0:2], z1[:, :, 2:4],
            )
            z2 = zt2[:].rearrange("p (cb l n) -> p cb l n", cb=CBLK, l=2)
            Zf = work.tile([128, CBLK * NPOS], F32, tag="Zf")
            nc.vector.tensor_add(
                Zf[:].rearrange("p (cb n) -> p cb n", cb=CBLK),
                z2[:, :, 0], z2[:, :, 1],
            )
            Zr = work.tile([128, CBLK * NPOS], F16, tag="Zr")
            with nc.allow_low_precision(reason="Z in [F/e, F*e]; fp16 1/Z fine"):
                nc.vector.reciprocal(Zr[:], Zf[:])
            pp = work.tile([128, NCHUNK * NPOS], F16, tag="pp")
            nc.vector.tensor_mul(
                pp[:].rearrange("p (i cb n) -> p i cb n", i=DI, cb=CBLK),
                pcv,
                Zr[:].rearrange("p (cb n) -> p cb n", cb=CBLK)
                .unsqueeze(1)
                .broadcast_to((128, DI, CBLK, NPOS)),
            )

            # ---------------- S + centroids (octet block-diag) -------------
            opre = work.tile([NPOS, F * DO], F32, tag="opre")
            y_sb = work.tile([NPOS, F * DO], F32, tag="y_sb")
            ppv = pp[:].rearrange("p (i cb n) -> p i cb n", i=DI, cb=CBLK)
            for O in range(NOCT):
                So = soct.tile([128, 8 * SF], F16, tag="So")
                for l in range(8):
                    eb = (
                        e16[:, O * CBLK * SLAB:(O + 1) * CBLK * SLAB]
                        .rearrange("p (cb l n) -> p cb l n", cb=CBLK, l=8)
                        [:, :, l, :]
                        .unsqueeze(1)
                        .broadcast_to((128, DI, CBLK, NPOS))
                    )
                    nc.vector.tensor_mul(
                        So[:, l * SF:(l + 1) * SF]
                        .rearrange("p (i cb n) -> p i cb n", i=DI, cb=CBLK),
                        ppv,
                        eb,
                    )
                cm = vps.tile([128, 1024], F32, tag="vh")
                Sv = So[:].rearrange("p (l x) -> p l x", l=8)
                for t in range(NCHUNK):
                    lhsT = w_r[:, t * F * DO + 128 * O:t * F * DO + 128 * (O + 1)]
                    for h in range(2):
                        nc.tensor.matmul(
                            cm[:, 512 * h:512 * h + 4 * NPOS]
                            .rearrange("p (l n) -> p l n", l=4),
                            lhsT,
                            Sv[:, 4 * h:4 * h + 4,
                               t * NPOS:(t + 1) * NPOS],
                            start=(t == 0),
                            stop=(t == NCHUNK - 1),
                        )
                # diagonal extraction: cen_oct[(l,o), pos] = cm[(l,o), (l,pos)]
                cmsb = work.tile([128, 8 * NPOS], F32, tag="cmsb")
                nc.scalar.copy(
                    cmsb[:].rearrange("p (h x) -> p h x", h=2),
                    cm[:].rearrange("p (h x) -> p h x", h=2)[:, :, 0:4 * NPOS],
                )
                cen_oct = work.tile([128, NPOS], F32, tag=f"cen{O}")
                for l in range(8):
                    src = cmsb[16 * l:16 * l + DO, l * NPOS:(l + 1) * NPOS]
                    dst = cen_oct[16 * l:16 * l + DO, :]
                    if l % 2 == 0:
                        nc.vector.tensor_copy(dst, src)
                    else:
                        nc.sync.dma_start(dst, src)
                tp2 = tps.tile([128, 128], F32, tag="tp")
                nc.tensor.transpose(tp2[0:NPOS, :], cen_oct[:], eye128f[:])
                nc.scalar.copy(opre[:, O * 128:(O + 1) * 128], tp2[0:NPOS, :])
                # per-octet squash2 (overlaps next octet's S/centroids)
                squash(
                    opre[:, O * 128:(O + 1) * 128]
                    .rearrange("p (f o) -> p f o", o=DO),
                    y_sb[:, O * 128:(O + 1) * 128]
                    .rearrange("p (f o) -> p f o", o=DO),
                    1.0,
                    f"sq2_{O}",
                    nf=8,
                )
                nc.sync.dma_start(
                    y_d[:, O * 128:(O + 1) * 128], y_sb[:, O * 128:(O + 1) * 128]
                )

    nc.compile()
    return nc


def _get_program():
    if "nc" not in _CACHE:
        _CACHE["nc"] = _build()
    return _CACHE["nc"]


def _in_maps(x, Wm):
    w_r, w8o, w8p, sel4 = _host_weights(Wm)
    eye72 = np.eye(NPOS, dtype=np.float32)
    eye128f = np.eye(128, dtype=np.float32)
    maps = []
    for k in range(8):
        p_ct, p_pk = _host_patches(x, k)
        maps.append({
            "p_ct": p_ct,
            "p_pk": p_pk,
            "w_r": w_r,
            "w8o": w8o,
            "w8p": w8p,
            "sel4": sel4,
            "eye72": eye72,
            "eye128f": eye128f,
        })
    return maps


def kernel(x, W):
    from concourse.bass_utils import run_bass_kernel_spmd

    x = np.asarray(x, np.float32)
    Wm = np.asarray(W, np.float32)[0, 0, 0]
    nc = _get_program()
    res = run_bass_kernel_spmd(nc, _in_maps(x, Wm), list(range(8)))
    Ho, Wo = H - KH + 1, WD - KW + 1
    y = np.empty((B, Ho, Wo, F, DO), np.float32)
    for k in range(8):
        b, hh = divmod(k, 2)
        y[b, 6 * hh:6 * hh + 6] = res.results[k]["y"].reshape(6, Wo, F, DO)
    return y
